# revision 5
# baseline (speedup 1.0000x reference)
"""Trainium2 Bass kernel for nn_Classifier (gnn_message_passing).

reference:
  V0,V1,V2 = V[F[...,0..2]]           (per-batch vertex gather)
  Cc=(V0+V1+V2)/3; N=0.5*cross(V1-V0, V2-V0); L=sqrt(clip(|N|^2,1e-6))
  x=[Cc, N/L]; h=sig(x@W1+b1); h=sig(h@W2+b2); h=h@W3+b3
  out = sum_faces(h*L)  -> [B, 40]

Strategy: data-parallel over B across 8 cores (4 batches/core). On each core:
 - F loaded to SBUF; vertex gather via SWDGE indirect DMA (V stays in DRAM).
 - DVE computes per-face features into a [128, PF, 8] buffer
   ([Csum, N/L, L, 0] per face; 1/3 folded into W1).
 - PE transposes 7-face groups -> [56,128] "xT" tiles (8 feats x 7 faces on
   partitions), block-diag W1 (7 blocks of [8,17], 17th unit = const-1 via
   bias 30 -> sigmoid=1 carries b2) -> z1 [119,128]; sigmoid+b1 on ACT.
 - L2 flipped: lhsT=h1-tile, rhs=block-diag [W2;b2] [119,448] -> z2 with
   faces on partitions; sigmoid on ACT.
 - weighted face-sum folded into PE: out[64,1] += h2[:,64t:].T @ L-col.
   Layer 3 is linear so it's applied once per batch on the reduced vector:
   out_b = [W3;b3].T @ [S; sum(L)].
Self-contained: hardcodes shapes; host side only shards/preps/stacks.
"""

import numpy as np

import concourse.bacc as bacc
import concourse.bass as bass
import concourse.mybir as mybir
import concourse.tile as tile
from concourse import bass_utils

F32 = mybir.dt.float32
I32 = mybir.dt.int32
ALU = mybir.AluOpType
AF = mybir.ActivationFunctionType

# full-size problem config
B, NV, NF, CCLS = 32, 50000, 100000, 40
NCORES = 8
NB = B // NCORES          # batches per core
PF = 784                  # faces per partition (128*784 = 100352 >= NF), mult of 7
CHUNK = 98                # faces/partition per gather chunk, mult of 7
NCHUNK = PF // CHUNK      # 8
BIGB = 30.0               # bias driving the const-one hidden unit


def _cfg_blocks(chunk):
    # per chunk: U face-columns of 7 faces each -> one [56,128] transpose each
    return chunk // 7


def build_nc(nb=NB, nv=NV, nf=NF, pf=PF, chunk=CHUNK):
    nchunk = pf // chunk
    U = _cfg_blocks(chunk)  # transposes per chunk
    nc = bacc.Bacc("TRN2", target_bir_lowering=False, debug=False)

    vd = nc.dram_tensor("V", [nb * nv, 3], F32, kind="ExternalInput").ap()
    fd = nc.dram_tensor("F", [nb, 3 * nf], I32, kind="ExternalInput").ap()
    w1d = nc.dram_tensor("W1blk", [56, 119], F32, kind="ExternalInput").ap()
    b1d = nc.dram_tensor("b1rep", [119, 1], F32, kind="ExternalInput").ap()
    w2d = nc.dram_tensor("W2blk", [119, 448], F32, kind="ExternalInput").ap()
    w3d = nc.dram_tensor("W3b", [65, CCLS], F32, kind="ExternalInput").ap()
    idd = nc.dram_tensor("ident", [128, 128], F32, kind="ExternalInput").ap()
    oned = nc.dram_tensor("ones", [128, 1], F32, kind="ExternalInput").ap()
    outd = nc.dram_tensor("OUT", [CCLS, nb], F32, kind="ExternalOutput").ap()

    with tile.TileContext(nc) as tc:
        with (
            tc.tile_pool(name="consts", bufs=1) as cpool,
            tc.tile_pool(name="fsb", bufs=1) as fpool,
            tc.tile_pool(name="gat", bufs=3) as gpool,
            tc.tile_pool(name="scr", bufs=2) as spool,
            tc.tile_pool(name="xts", bufs=2) as xpool,
            tc.tile_pool(name="h1p", bufs=2) as h1pool,
            tc.tile_pool(name="h2p", bufs=3) as h2pool,
            tc.tile_pool(name="xtp", bufs=2, space="PSUM") as xtpsum,
            tc.tile_pool(name="z1p", bufs=2, space="PSUM") as z1psum,
            tc.tile_pool(name="z2p", bufs=2, space="PSUM") as z2psum,
            tc.tile_pool(name="accp", bufs=1, space="PSUM") as accpsum,
        ):
            w1t = cpool.tile([56, 119], F32)
            b1t = cpool.tile([119, 1], F32)
            w2t = cpool.tile([119, 448], F32)
            w3t = cpool.tile([65, CCLS], F32)
            idt = cpool.tile([128, 128], F32)
            onet = cpool.tile([128, 1], F32)
            outsb = cpool.tile([CCLS, nb], F32)
            feat = cpool.tile([128, pf, 8], F32)
            fsb = fpool.tile([128, pf, 3], I32)

            nc.sync.dma_start(w1t[:], w1d[:])
            nc.sync.dma_start(b1t[:], b1d[:])
            nc.sync.dma_start(w2t[:], w2d[:])
            nc.sync.dma_start(w3t[:], w3d[:])
            nc.sync.dma_start(idt[:], idd[:])
            nc.sync.dma_start(onet[:], oned[:])
            nc.gpsimd.memset(feat[:], 0.0)
            nc.gpsimd.memset(fsb[:], 0)

            for b in range(nb):
                # ---- load F[b] into [128, pf, 3] (flat row-major split) ----
                n_full = (3 * nf) // (3 * pf)          # partitions fully covered
                fsb_flat = fsb[:].rearrange("p a b -> p (a b)")
                nc.sync.dma_start(
                    fsb_flat[0:n_full, :],
                    fd[b : b + 1, 0 : n_full * 3 * pf].rearrange(
                        "o (p q) -> (o p) q", p=n_full
                    ),
                )
                rem = 3 * nf - n_full * 3 * pf
                if rem:
                    nc.sync.dma_start(
                        fsb_flat[n_full : n_full + 1, 0:rem],
                        fd[b : b + 1, n_full * 3 * pf : 3 * nf],
                    )

                acc = accpsum.tile([65, 8], F32)
                first = True
                for c in range(nchunk):
                    # ---- gather all 3 corners of CHUNK faces/partition ----
                    vg = gpool.tile([128, chunk, 3, 3], F32, tag="vg")
                    nc.gpsimd.indirect_dma_start(
                        out=vg[:].rearrange("p a b c -> p (a b c)"),
                        out_offset=None,
                        in_=vd[:],
                        in_offset=bass.IndirectOffsetOnAxis(
                            ap=fsb[:, c * chunk : (c + 1) * chunk, :], axis=0
                        ),
                        element_offset=b * nv * 3,
                    )
                    v0 = vg[:, :, 0, :]
                    v1 = vg[:, :, 1, :]
                    v2 = vg[:, :, 2, :]
                    fsl = feat[:, c * chunk : (c + 1) * chunk, :]
                    csum = fsl[:, :, 0:3]
                    nhat = fsl[:, :, 3:6]
                    lcol = fsl[:, :, 6:7]
                    # centroid sum (x3; 1/3 folded into W1)
                    nc.vector.tensor_add(csum, v0, v1)
                    nc.vector.tensor_add(csum, csum, v2)
                    # edges
                    e1 = spool.tile([128, chunk, 3], F32, tag="e1")
                    e2 = spool.tile([128, chunk, 3], F32, tag="e2")
                    nc.vector.tensor_sub(e1[:], v1, v0)
                    nc.vector.tensor_sub(e2[:], v2, v0)
                    # cross product -> nhat slots (x0.5 folded: N=0.5*cross;
                    # 0.5 and 1/L cancel in N/L; but L=|0.5*cross| -> fold
                    # 0.25 into the square-sum below)
                    t1 = spool.tile([128, chunk], F32, tag="t1")
                    t2 = spool.tile([128, chunk], F32, tag="t2")
                    for k, (a, bb) in enumerate(((1, 2), (2, 0), (0, 1))):
                        nc.vector.tensor_mul(t1[:], e1[:, :, a], e2[:, :, bb])
                        nc.vector.tensor_mul(t2[:], e1[:, :, bb], e2[:, :, a])
                        nc.vector.tensor_sub(fsl[:, :, 3 + k], t1[:], t2[:])
                    # |N|^2 = 0.25 * sum(cross^2), clip, 1/sqrt
                    n2 = spool.tile([128, chunk, 3], F32, tag="n2")
                    nc.vector.tensor_mul(n2[:], nhat, nhat)
                    l2 = spool.tile([128, chunk], F32, tag="l2")
                    nc.vector.tensor_reduce(
                        l2[:], n2[:], axis=mybir.AxisListType.X, op=ALU.add
                    )
                    m = spool.tile([128, chunk], F32, tag="m")
                    nc.vector.tensor_scalar(
                        m[:], l2[:], 0.25, 1e-6, op0=ALU.mult, op1=ALU.max
                    )
                    inv = spool.tile([128, chunk], F32, tag="inv")
                    nc.vector.reciprocal(inv[:], m[:])
                    q = spool.tile([128, chunk], F32, tag="q")
                    nc.scalar.sqrt(q[:], inv[:])  # q = 1/sqrt(m)
                    # L = m*q = sqrt(m); nhat = 0.5*cross*q (0.5*q since the
                    # cross stored unscaled and N/L needs the same scale up/dn
                    # -> actually N/L = cross/|cross| (scale-free); L needs 0.5)
                    nc.vector.tensor_mul(
                        lcol,
                        m[:].rearrange("p (a o) -> p a o", o=1),
                        q[:].rearrange("p (a o) -> p a o", o=1),
                    )
                    nc.vector.tensor_mul(
                        nhat, nhat, q[:].rearrange("p (a o) -> p a o", o=1).to_broadcast([128, chunk, 3])
                    )
                    # ---- transpose + MLP per 7-face column group ----
                    for u4 in range(0, U, 4):
                        g = min(4, U - u4)
                        xtp = xtpsum.tile([56, 512], F32, tag="xtp")
                        for i in range(g):
                            u = u4 + i
                            nc.tensor.transpose(
                                xtp[:, 128 * i : 128 * (i + 1)],
                                fsl[:, 7 * u : 7 * (u + 1), :].rearrange(
                                    "p a b -> p (a b)"
                                ),
                                idt[:],
                            )
                        xts = xpool.tile([56, 512], F32, tag="xts")
                        nc.scalar.copy(xts[:, 0 : 128 * g], xtp[:, 0 : 128 * g])
                        z1 = z1psum.tile([119, 512], F32, tag="z1")
                        for i in range(g):
                            nc.tensor.matmul(
                                z1[:, 128 * i : 128 * (i + 1)],
                                lhsT=w1t[:],
                                rhs=xts[:, 128 * i : 128 * (i + 1)],
                                start=True,
                                stop=True,
                            )
                        h1 = h1pool.tile([119, 512], F32, tag="h1")
                        nc.scalar.activation(
                            h1[:, 0 : 128 * g],
                            z1[:, 0 : 128 * g],
                            AF.Sigmoid,
                            bias=b1t[:],
                        )
                        for i in range(g):
                            u = u4 + i
                            z2 = z2psum.tile([128, 448], F32, tag="z2")
                            nc.tensor.matmul(
                                z2[:],
                                lhsT=h1[:, 128 * i : 128 * (i + 1)],
                                rhs=w2t[:],
                                start=True,
                                stop=True,
                            )
                            h2 = h2pool.tile([128, 448], F32, tag="h2")
                            nc.scalar.activation(h2[:], z2[:], AF.Sigmoid)
                            for t in range(7):
                                nc.tensor.matmul(
                                    acc[0:64, 0:1],
                                    lhsT=h2[:, 64 * t : 64 * (t + 1)],
                                    rhs=fsl[:, 7 * u + t : 7 * u + t + 1, 6],
                                    start=first,
                                    stop=(
                                        c == nchunk - 1
                                        and u == U - 1
                                        and t == 6
                                    ),
                                    skip_group_check=True,
                                )
                                if first:
                                    first = False
                            nc.tensor.matmul(
                                acc[64:65, 0:7],
                                lhsT=onet[:],
                                rhs=fsl[:, 7 * u : 7 * (u + 1), 6],
                                start=(c == 0 and u == 0),
                                stop=(c == nchunk - 1 and u == U - 1),
                                skip_group_check=True,
                            )
                # ---- batch epilogue: out_b = [W3;b3].T @ [S; sumL] ----
                rhs65 = spool.tile([65, 1], F32, tag="rhs65")
                nc.vector.tensor_copy(rhs65[0:64, :], acc[0:64, 0:1])
                nc.vector.tensor_reduce(
                    rhs65[64:65, :], acc[64:65, 0:7],
                    axis=mybir.AxisListType.X, op=ALU.add,
                )
                outp = accpsum.tile([CCLS, 1], F32, tag="outp")
                nc.tensor.matmul(
                    outp[:], lhsT=w3t[:], rhs=rhs65[:], start=True, stop=True
                )
                nc.vector.tensor_copy(outsb[:, b : b + 1], outp[:])
            nc.sync.dma_start(outd[:], outsb[:])
    nc.compile()
    return nc


def prep_weights(W1, b1, W2, b2, W3, b3):
    W1blk = np.zeros((56, 119), np.float32)
    W1p = np.zeros((8, 17), np.float32)
    W1p[0:3, 0:16] = W1[0:3] / 3.0
    # feat stores cross*q = 2*(N/L) exactly (N=0.5*cross, L=sqrt(m)) -> 0.5 here
    W1p[3:6, 0:16] = W1[3:6] / 2.0
    b1rep = np.zeros((119, 1), np.float32)
    for t in range(7):
        W1blk[8 * t : 8 * t + 8, 17 * t : 17 * t + 17] = W1p
        b1rep[17 * t : 17 * t + 16, 0] = b1
        b1rep[17 * t + 16, 0] = BIGB
    W2blk = np.zeros((119, 448), np.float32)
    w2e = np.vstack([W2, b2[None, :]]).astype(np.float32)  # [17, 64]
    for t in range(7):
        W2blk[17 * t : 17 * t + 17, 64 * t : 64 * t + 64] = w2e
    W3b = np.vstack([W3, b3[None, :]]).astype(np.float32)  # [65, 40]
    return W1blk, b1rep, W2blk, W3b


_NC_CACHE = {}


def kernel(V, F, W1, b1, W2, b2, W3, b3):
    V = np.ascontiguousarray(np.asarray(V, np.float32))
    F = np.ascontiguousarray(np.asarray(F, np.int32))
    W1blk, b1rep, W2blk, W3b = prep_weights(
        np.asarray(W1, np.float32), np.asarray(b1, np.float32),
        np.asarray(W2, np.float32), np.asarray(b2, np.float32),
        np.asarray(W3, np.float32), np.asarray(b3, np.float32),
    )
    ident = np.eye(128, dtype=np.float32)
    ones = np.ones((128, 1), np.float32)
    if "nc" not in _NC_CACHE:
        _NC_CACHE["nc"] = build_nc()
    nc = _NC_CACHE["nc"]
    in_maps = []
    for c in range(NCORES):
        in_maps.append({
            "V": V[c * NB : (c + 1) * NB].reshape(NB * NV, 3),
            "F": F[c * NB : (c + 1) * NB].reshape(NB, 3 * NF),
            "W1blk": W1blk, "b1rep": b1rep, "W2blk": W2blk, "W3b": W3b,
            "ident": ident, "ones": ones,
        })
    res = bass_utils.run_bass_kernel_spmd(nc, in_maps, core_ids=list(range(NCORES)))
    out = np.concatenate([res.results[c]["OUT"].T for c in range(NCORES)], axis=0)
    return out.astype(np.float32)


if __name__ == "__main__":
    pass
